# revision 3
# baseline (speedup 1.0000x reference)
"""MoE (mxfp4, top-2 routing) Trainium2 kernel — token-sparse flipped matmuls.

Sharding: expert-parallel, 2 experts per core. Weights are kept exact in
fp8e5m2 (every mxfp4 value * e8m0 scale is exactly representable) and are
streamed through the PE's LDWEIGHTS port as 128x128 stationary tiles, which
(with the hardware fast-weight-load path for 128-column fp8 tiles) moves
weights ~1.65x faster than streaming them as the moving operand. The moving
operand is the small set of routed tokens per expert (top-2 routing => ~16
tokens/expert, padded to a fixed capacity), so matmul column counts stay at
or below the weight-load time and the PE runs at the weight-load rate.

Per expert: gu^T tiles [feat,128 x tok] accumulate over 8 H-chunks; silu
(+bias via ACT bias operand) and up-bias+mul produce h^T directly in
[feat, tok] layout (no transposes); down matmuls contract F the same way.
Unscaled per-expert y^T goes back to the host, which applies the top-2
combine weights, scatters to token rows, and adds the bias_down term
(combine @ bias_down) in f32.

Device inputs per core: x gathered per expert (bf16), gu biases (f32),
weights (e5m2). Everything is SBUF-resident (no buffer reuse). Weights
stream on the single sync HWDGE queue in PE consumption order (gu0, gu1,
dn0, dn1) as contiguous chunks that shrink from 2MB to 256KB toward the
stream end, so the stream stays near the ~330GB/s HBM rate while the PE
finishes right behind the last bytes.
"""

import sys
import numpy as np

for _p in ("/opt/trn_rl_repo", "/root/.axon_site/_ro/trn_rl_repo"):
    if _p not in sys.path:
        sys.path.insert(0, _p)

import ml_dtypes

FP4_LUT = np.array(
    [0.0, 0.5, 1.0, 1.5, 2.0, 3.0, 4.0, 6.0,
     -0.0, -0.5, -1.0, -1.5, -2.0, -3.0, -4.0, -6.0],
    dtype=np.float32,
)
BLOCK = 32
E, H, F, T = 16, 1024, 2048, 128
N_CORES = 8
EXP_PER_CORE = E // N_CORES

BF16 = ml_dtypes.bfloat16
E5M2 = ml_dtypes.float8_e5m2

_compiled = {}

WGU_CHUNKS = [[(0, 8), (8, 16)], [(0, 8), (8, 12), (12, 16)]]
WDN_CHUNKS = [[(0, 4), (4, 8)], [(0, 4), (4, 6), (6, 7), (7, 8)]]


def _dequant(blocks, scales):
    b = blocks.astype(np.uint8)
    lo = b & 0xF
    hi = (b >> 4) & 0xF
    nib = np.stack([lo, hi], axis=-1).reshape(blocks.shape[:-1] + (blocks.shape[-1] * 2,))
    vals = FP4_LUT[nib]
    s = np.exp2(scales.astype(np.float32) - 127.0)
    s = np.repeat(s, BLOCK, axis=-1)
    return vals * s


def _build(cap):
    from concourse import bacc, mybir, tile

    f32 = mybir.dt.float32
    bf16 = mybir.dt.bfloat16
    e5 = mybir.dt.float8e5

    nc = bacc.Bacc(
        "TRN2", target_bir_lowering=False, debug=False, num_devices=N_CORES
    )

    xg_d = nc.declare_dram_parameter("xg", [128, 2, 8, cap], bf16, isOutput=False)
    bgu_d = nc.declare_dram_parameter("bgu", [128, 2, 2, 16], f32, isOutput=False)
    # weights arrive as one contiguous DRAM region per DMA chunk
    wgu_d = [
        [nc.declare_dram_parameter(f"wgu{e}c{c}", [128, (b - a) * 2048], e5,
                                   isOutput=False)
         for c, (a, b) in enumerate(WGU_CHUNKS[e])]
        for e in range(EXP_PER_CORE)
    ]
    wdn_d = [
        [nc.declare_dram_parameter(f"wdn{e}c{c}", [128, (b - a) * 2048], e5,
                                   isOutput=False)
         for c, (a, b) in enumerate(WDN_CHUNKS[e])]
        for e in range(EXP_PER_CORE)
    ]
    y_d = [
        nc.declare_dram_parameter(f"y{e}", [128, 8, cap], f32, isOutput=True)
        for e in range(EXP_PER_CORE)
    ]

    AF = mybir.ActivationFunctionType
    OP = mybir.AluOpType

    with tile.TileContext(nc) as tc:
        with (
            tc.tile_pool(name="const", bufs=1) as constp,
            tc.tile_pool(name="work", bufs=3) as workp,
            tc.tile_pool(name="psgu", bufs=4, space="PSUM") as ps_gu,
            tc.tile_pool(name="psy", bufs=2, space="PSUM") as ps_yp,
        ):
            xg = constp.tile([128, 2, 8, cap], bf16, tag="xg")
            nc.gpsimd.dma_start(out=xg[:], in_=xg_d[:])
            bgu = constp.tile([128, 2, 2, 16], f32, tag="bgu")
            nc.gpsimd.dma_start(out=bgu[:], in_=bgu_d[:])

            wgu_t = []
            wdn_t = []
            h_t = []
            y_t = []
            for e in range(EXP_PER_CORE):
                wgu_t.append(constp.tile([128, 16, 8, 2, 128], e5, name=f"wgu{e}", tag=f"wgu{e}"))
                wdn_t.append(constp.tile([128, 8, 16, 128], e5, name=f"wdn{e}", tag=f"wdn{e}"))
                h_t.append(constp.tile([128, 16, cap], bf16, name=f"h{e}", tag=f"h{e}"))
                y_t.append(constp.tile([128, 8, cap], f32, name=f"y{e}", tag=f"y{e}"))

            # weight stream on the sync HWDGE queue, in PE consumption order
            # gu0, gu1, dn0, dn1; chunks shrink toward the end of the stream
            # so the PE finishes right behind it.
            for e in range(EXP_PER_CORE):
                for c, (a, b) in enumerate(WGU_CHUNKS[e]):
                    nc.sync.dma_start(out=wgu_t[e][:, a:b], in_=wgu_d[e][c][:])
            for e in range(EXP_PER_CORE):
                for c, (a, b) in enumerate(WDN_CHUNKS[e]):
                    nc.sync.dma_start(out=wdn_t[e][:, a:b], in_=wdn_d[e][c][:])

            def gu_round(e, r):
                xge = xg[:, e]  # [128, 8, cap]
                ps_g = ps_gu.tile([128, cap], f32, tag="ps", name="ps")
                ps_u = ps_gu.tile([128, cap], f32, tag="ps", name="ps")
                for kh in range(8):
                    nc.tensor.matmul(
                        ps_g[:], wgu_t[e][:, r, kh, 0, :], xge[:, kh, :],
                        start=(kh == 0), stop=(kh == 7),
                    )
                for kh in range(8):
                    nc.tensor.matmul(
                        ps_u[:], wgu_t[e][:, r, kh, 1, :], xge[:, kh, :],
                        start=(kh == 0), stop=(kh == 7),
                    )
                sil = workp.tile([128, cap], f32, tag="sil", name="sil")
                nc.scalar.activation(
                    sil[:], ps_g[:], AF.Silu,
                    bias=bgu[:, e, 0, r:r + 1], scale=1.0,
                )
                upb = workp.tile([128, cap], f32, tag="upb", name="upb")
                nc.vector.tensor_scalar(
                    upb[:], ps_u[:], bgu[:, e, 1, r:r + 1], None, op0=OP.add
                )
                nc.vector.tensor_tensor(
                    h_t[e][:, r, :], sil[:], upb[:], op=OP.mult
                )

            def dn_round(e, hh):
                psy = ps_yp.tile([128, cap], f32, tag="psy", name="psy")
                for kf in range(16):
                    nc.tensor.matmul(
                        psy[:], wdn_t[e][:, hh, kf, :], h_t[e][:, kf, :],
                        start=(kf == 0), stop=(kf == 15),
                    )
                nc.vector.tensor_copy(y_t[e][:, hh, :], psy[:])
                if hh == 3:
                    nc.sync.dma_start(out=y_d[e][:, 0:4], in_=y_t[e][:, 0:4])
                elif hh == 7:
                    nc.sync.dma_start(out=y_d[e][:, 4:8], in_=y_t[e][:, 4:8])

            for e in range(EXP_PER_CORE):
                for r in range(16):
                    gu_round(e, r)
            for e in range(EXP_PER_CORE):
                for hh in range(8):
                    dn_round(e, hh)

    nc.finalize()
    return nc


def _prep(hidden_states, router_w, bias_gu, bias_down,
          blocks_gu, scales_gu, blocks_down, scales_down):
    x = np.asarray(hidden_states, dtype=np.float32).reshape(T, H)

    # host router: logits -> top-2 -> softmax -> combine weights
    logits = x @ np.asarray(router_w, dtype=np.float32).T
    order = np.argsort(-logits, axis=-1, kind="stable")
    i1, i2 = order[:, 0], order[:, 1]
    v1 = logits[np.arange(T), i1]
    v2 = logits[np.arange(T), i2]
    w1 = 1.0 / (1.0 + np.exp(v2 - v1))
    w2 = 1.0 - w1
    combine = np.zeros((T, E), dtype=np.float32)
    combine[np.arange(T), i1] = w1
    combine[np.arange(T), i2] = w2

    idx_lists = [np.where(combine[:, ge] != 0.0)[0] for ge in range(E)]
    maxcnt = max(len(ix) for ix in idx_lists)
    cap = 32
    while cap < maxcnt:
        cap *= 2
    cap = min(cap, 128)

    w_gu = _dequant(np.asarray(blocks_gu), np.asarray(scales_gu))      # [E, 4096, 1024]
    w_dn = _dequant(np.asarray(blocks_down), np.asarray(scales_down))  # [E, 1024, 2048]
    bias_gu = np.asarray(bias_gu, dtype=np.float32)

    in_maps = []
    for core in range(N_CORES):
        m = {}
        xg = np.zeros((128, EXP_PER_CORE, 8, cap), dtype=BF16)
        bg = np.zeros((128, EXP_PER_CORE, 2, 16), dtype=np.float32)
        for e in range(EXP_PER_CORE):
            ge = core * EXP_PER_CORE + e
            idx = idx_lists[ge]
            xs = x[idx].astype(BF16)                      # [n, 1024]
            xs = xs.reshape(len(idx), 8, 128).transpose(2, 1, 0)  # [p, kh, t]
            xg[:, e, :, :len(idx)] = xs
            bg[:, e, :, :] = bias_gu[ge].reshape(2, 16, 128).transpose(2, 0, 1)

            wg = w_gu[ge]                                  # [4096, 1024]
            gate = wg[:2048].reshape(16, 128, 8, 128)      # r, m, kh, p
            up = wg[2048:].reshape(16, 128, 8, 128)
            st = np.stack([gate, up], axis=0)              # i, r, m, kh, p
            wgu_sb = np.ascontiguousarray(
                st.transpose(4, 1, 3, 0, 2)).astype(E5M2)  # [p, r, kh, i, m]
            for c, (a, b) in enumerate(WGU_CHUNKS[e]):
                m[f"wgu{e}c{c}"] = np.ascontiguousarray(
                    wgu_sb[:, a:b].reshape(128, (b - a) * 2048))

            wd = w_dn[ge].reshape(8, 128, 16, 128)          # hh, m, kf, p
            wdn_sb = np.ascontiguousarray(
                wd.transpose(3, 0, 2, 1)).astype(E5M2)      # [p, hh, kf, m]
            for c, (a, b) in enumerate(WDN_CHUNKS[e]):
                m[f"wdn{e}c{c}"] = np.ascontiguousarray(
                    wdn_sb[:, a:b].reshape(128, (b - a) * 2048))
        m["xg"] = xg
        m["bgu"] = bg
        in_maps.append(m)
    return in_maps, combine, idx_lists, cap


def kernel(hidden_states, router_w, bias_gu, bias_down,
           blocks_gu, scales_gu, blocks_down, scales_down, _trace=False):
    from concourse.bass_utils import run_bass_kernel_spmd

    in_maps, combine, idx_lists, cap = _prep(
        hidden_states, router_w, bias_gu, bias_down,
        blocks_gu, scales_gu, blocks_down, scales_down)

    if cap not in _compiled:
        _compiled[cap] = _build(cap)
    nc = _compiled[cap]

    res = run_bass_kernel_spmd(nc, in_maps, list(range(N_CORES)), trace=_trace)

    out = np.zeros((T, H), dtype=np.float32)
    for core in range(N_CORES):
        om = res.results[core]
        for e in range(EXP_PER_CORE):
            ge = core * EXP_PER_CORE + e
            idx = idx_lists[ge]
            yT = np.asarray(om[f"y{e}"], dtype=np.float32)  # [128, 8, cap]
            yv = yT[:, :, :len(idx)]                        # [p, hh, t]
            y = yv.transpose(2, 1, 0).reshape(len(idx), H)  # [t, hh*128]
            out[idx] += combine[idx, ge][:, None] * y
    out += combine @ np.asarray(bias_down, dtype=np.float32)
    out = out.reshape(1, T, H)
    if _trace:
        return out, res
    return out
